# revision 23
# baseline (speedup 1.0000x reference)
"""Trainium2 Bass kernel for a WaveNet-style dilated-conv stack (v3).

Network (per reference):
  x1 = conv1d(x, Wc, bc, d=1, pad=1)                      # 1 -> 32, host-side (exact fp32)
  for l in 27 layers, d = 2^(l%9):
      g = tanh(conv(x, Wt_l, d)) * sigmoid(conv(x, Ws_l, d))   # 32->32, k=3, pad=d
      skip += conv1x1(g, Wskip_l)                              # 32->512
      x = conv1x1(g, Wdense_l) + x
  out = conv1x1(relu(conv1x1(skip, Wp1)), Wp2)            # 512->512->256
  return log_softmax(out, axis=channels)

Device strategy (8 cores, sequence-parallel, no cross-core comm):
  - Wp1 folded into skip weights on host (W1s = Wp1 @ Wskip).
  - Each core owns 16384 steps as 2 halves of 8192 with a 2048-step halo
    (receptive radius 1533 < 2048); window = 12288 computed cols = 6 units
    of 2048.
  - x is stored with its two dilation-shifted tap replicas on partition
    strips {0:32 (x>>d), 32:64 (x), 64:96 (x<<d)}, maintained by SBUF->SBUF
    DMA after each layer's x update.  Each gated conv is then ONE K=96
    matmul per (function, strip): 8 matmuls per 2048-col unit, col-tiled
    M=32 so tanh lands on psum cols 0:512 / sigmoid on 512:1024 with the 4
    strips stacked over 128 partitions -> 128-lane activations.
  - g = tanh*sig computed stacked [128, 512]; scattered to the flat ring
    by 4 SBUF->SBUF DMAs.
  - Two ring panels of 4 layers each; skip matmuls accumulate 2 groups
    (8 layers) per PSUM tile before one DVE add into h (relu bias folded
    into h's first write; h laid out [128, cb*2048+m*512] so the post
    relu is one 2048-wide op).
  - x update is IN PLACE on strip 32:64: even layers on DVE via
    scalar_tensor_tensor((pd+bdense)+x_old); odd layers residual via
    identity matmul on the PE and ScalarE Identity+bias evacuation.
  - Post: exp biased, partition-sum via ones-matmul, ln, broadcast via
    -1s matmul, final add on DVE with one PSUM operand.
"""

import numpy as np
import ml_dtypes

BF16 = ml_dtypes.bfloat16

DIL = [2 ** i for i in range(9)] * 3
L = len(DIL)            # 27
RD, SD, QD = 32, 512, 256
T = 131072
NCORES = 8
V = T // NCORES         # 16384 per core
VH = V // 2             # 8192 per half
HALO = 1536             # >= 1533 total dilation radius
PAD = 256               # >= max dilation
WH = VH + 2 * HALO      # 11264 computed window per half
WA = WH + 2 * PAD       # 11776 allocated width per half
VOFF = HALO + PAD       # 1792 valid-region offset inside window
# units: 5 full 2048-col units + one 1024-col tail (strip-aligned)
UNITS = [(PAD + 2048 * k, 2048, 4) for k in range(5)] + [(PAD + 10240, 1024, 2)]
SV0 = (VOFF - PAD) // 512        # first valid 512-strip index (3)
SV1 = SV0 + VH // 512 - 1        # last valid strip index (18)
NB = VH // 512          # 16 valid 512-col blocks per half
NGRP = (L + 3) // 4     # 7 groups of (up to) 4 layers

_cache = {}
_last_run = {}


def _build():
    from contextlib import ExitStack

    import concourse.bacc as bacc
    import concourse.mybir as mybir
    import concourse.tile as tile

    dt = mybir.dt
    AF = mybir.ActivationFunctionType
    ALU = mybir.AluOpType
    f32, bf16 = dt.float32, dt.bfloat16

    nc = bacc.Bacc("TRN2", target_bir_lowering=False, debug=False,
                   num_devices=NCORES)

    def din(name, shape, dty):
        return nc.dram_tensor(name, shape, dty, kind="ExternalInput").ap()

    xin_d = din("xin", [96, 2 * WA], bf16)
    wgk_d = din("wgk", [96, L * 2 * 32], bf16)   # 3-tap lhsT per (l, fn)
    wdx_d = din("wdx", [128, L * 4 * 64], bf16)  # dense lhsT per (l, strip), M=64
    idw_d = din("idw", [128, 2 * RD], bf16)      # residual identity lhsT, M=64
    idh_d = din("idh", [128, 128], bf16)         # h-accumulate identity lhsT
    wskp_d = din("wskp", [128, NGRP * 4 * 128], bf16)  # skip lhsT per (grp, m)
    wp2_d = din("wp2", [128, 8 * 128], bf16)     # Wp2 lhsT per (q, p)
    btt_d = din("btt", [128, L], f32)            # bt stacked x4
    bst_d = din("bst", [128, L], f32)            # bs stacked x4
    bdc_d = din("bdc", [64, L], f32)             # bdense (2 strips)
    hb_d = din("hb", [128, 4], f32)
    bp2c_d = din("bp2c", [128, 2], f32)
    sumw_d = din("sumw", [128, 1], f32)
    nones_d = din("nones", [1, 128], f32)
    out_d = nc.dram_tensor("out", [QD, V], f32, kind="ExternalOutput").ap()

    with tile.TileContext(nc) as tc, ExitStack() as top:
        wp = top.enter_context(tc.tile_pool(name="wp", bufs=1))

        def load(d, tag):
            t = wp.tile(list(d.shape), d.dtype, tag=tag, name=tag)
            nc.sync.dma_start(t[:], d[:])
            return t

        wgk = load(wgk_d, "wgk")
        wdx = load(wdx_d, "wdx")
        idw = load(idw_d, "idw")
        idh = load(idh_d, "idh")
        wskp = load(wskp_d, "wskp")
        wp2 = load(wp2_d, "wp2")
        btt = load(btt_d, "btt")
        bst = load(bst_d, "bst")
        bdc = load(bdc_d, "bdc")
        hb = load(hb_d, "hb")
        bp2c = load(bp2c_d, "bp2c")
        sumw = load(sumw_d, "sumw")
        nones = load(nones_d, "nones")

        # x + shifted tap replicas; rows 96:128 stay zero for K=128 matmuls
        xx = wp.tile([128, WA], bf16, tag="xx", name="xx")
        ringA = wp.tile([128, WA], bf16, tag="ringA", name="ringA")
        ringB = wp.tile([128, WA], bf16, tag="ringB", name="ringB")
        rings = [ringA, ringB]
        h = wp.tile([128, 4 * VH], bf16, tag="h", name="h")
        nc.vector.memset(xx[:], 0.0)
        nc.gpsimd.memset(ringA[:], 0.0)
        nc.gpsimd.memset(ringB[:], 0.0)

        for half in range(2):
            nc.sync.dma_start(xx[0:96, :], xin_d[:, half * WA:(half + 1) * WA])
            with ExitStack() as lctx:
                pg = lctx.enter_context(
                    tc.tile_pool(name=f"pg{half}", bufs=2, space="PSUM"))
                pd = lctx.enter_context(
                    tc.tile_pool(name=f"pd{half}", bufs=2, space="PSUM"))
                tu = lctx.enter_context(tc.tile_pool(name=f"tu{half}", bufs=4))

                # software pipeline: the dense conv + x update + replica DMAs
                # of unit (l, u) are emitted AFTER unit (l, u+1)'s gated
                # front, so no engine queue head-of-line blocks on the
                # gated->act->mul->scatter chain of the same unit.
                pending = []

                def emit_back(l2, b2, w_, gm2):
                    if l2 == L - 1:
                        return      # x_{27} is dead: skip dense + update
                    on_dve = ((b2 - PAD) // 2048 + 5 * l2) % 8 < 5
                    for v2 in range(w_ // 1024):
                        c0 = b2 + v2 * 1024
                        # rows 32:64 of a [64,...] psum tile so the DVE stt
                        # sees matching start partitions
                        pdt = pd.tile([64, 1024], f32, tag="pd", name="pd")
                        for w2 in range(2):
                            s2 = v2 * 2 + w2
                            nc.tensor.matmul(
                                pdt[0:64, w2 * 512:(w2 + 1) * 512],
                                wdx[:, (l2 * 4 + s2) * 64:
                                    (l2 * 4 + s2 + 1) * 64],
                                gm2[:, 0:512],
                                start=True, stop=on_dve,
                                tile_position=(0, 0))
                        if not on_dve:
                            for w2 in range(2):
                                nc.tensor.matmul(
                                    pdt[0:64, w2 * 512:(w2 + 1) * 512],
                                    idw[:],
                                    xx[:, c0 + w2 * 512:c0 + (w2 + 1) * 512],
                                    start=False, stop=True,
                                    tile_position=(0, 0))
                            nc.scalar.activation(
                                xx[RD:2 * RD, c0:c0 + 1024],
                                pdt[RD:2 * RD, :],
                                AF.Identity, bias=bdc[RD:2 * RD, l2:l2 + 1])
                        else:
                            nc.vector.scalar_tensor_tensor(
                                xx[RD:2 * RD, c0:c0 + 1024],
                                pdt[RD:2 * RD, :],
                                bdc[RD:2 * RD, l2:l2 + 1],
                                xx[RD:2 * RD, c0:c0 + 1024],
                                op0=ALU.add, op1=ALU.add)
                    dn = DIL[l2 + 1]
                    nc.sync.dma_start(
                        xx[0:RD, b2 + dn:b2 + w_ + dn],
                        xx[RD:2 * RD, b2:b2 + w_])
                    nc.gpsimd.dma_start(
                        xx[2 * RD:3 * RD, b2 - dn:b2 + w_ - dn],
                        xx[RD:2 * RD, b2:b2 + w_])

                for l in range(L):
                    j = l % 4
                    G = l // 4
                    ring = rings[G % 2]
                    for b0, w, ns in UNITS:
                        pgt = pg.tile([128, 1024], f32, tag="pg", name="pg")
                        for s in range(ns):
                            c0 = b0 + s * 512
                            for f in range(2):
                                nc.tensor.matmul(
                                    pgt[32 * s:32 * (s + 1),
                                        512 * f:512 * f + 512],
                                    wgk[0:96,
                                        (l * 2 + f) * 32:(l * 2 + f + 1) * 32],
                                    xx[0:96, c0:c0 + 512],
                                    start=True, stop=True,
                                    tile_position=(0, 32 * s))
                        pp = 32 * ns
                        gt = tu.tile([128, 512], bf16, tag="gt", name="gt")
                        gs = tu.tile([128, 512], bf16, tag="gs", name="gs")
                        gm = tu.tile([128, 512], bf16, tag="gm", name="gm")
                        nc.scalar.activation(gt[0:pp, :], pgt[0:pp, 0:512],
                                             AF.Tanh, bias=btt[0:pp, l:l + 1])
                        nc.scalar.activation(gs[0:pp, :], pgt[0:pp, 512:1024],
                                             AF.Sigmoid,
                                             bias=bst[0:pp, l:l + 1])
                        nc.vector.tensor_mul(gm[0:pp, :], gt[0:pp, :],
                                             gs[0:pp, :])
                        for s in range(ns):
                            gsi = (b0 - PAD) // 512 + s
                            if SV0 <= gsi <= SV1:
                                nc.vector.tensor_copy(
                                    ring[RD * j:RD * (j + 1),
                                         b0 + 512 * s:b0 + 512 * (s + 1)],
                                    gm[32 * s:32 * (s + 1), :])
                        pending.append((l, b0, w, gm))
                        if len(pending) > 2:
                            emit_back(*pending.pop(0))

                    if (j == 3 and G % 2 == 1) or l == L - 1:
                        while pending:
                            emit_back(*pending.pop(0))
                    if j == 3 or l == L - 1:
                        pidx = G // 2
                        if G % 2 == 1 or l == L - 1:
                            gA = 2 * pidx
                            grps = [g for g in (gA, gA + 1) if g <= G]
                            for m in range(4):
                                for cb in range(NB):
                                    c0 = VOFF + cb * 512
                                    pst = pg.tile([128, 512], f32, tag="pg",
                                                  name="pg")
                                    hcol = cb * 2048 + m * 512
                                    for gi, g in enumerate(grps):
                                        nc.tensor.matmul(
                                            pst[:],
                                            wskp[:, (g * 4 + m) * 128:
                                                 (g * 4 + m + 1) * 128],
                                            rings[g % 2][:, c0:c0 + 512],
                                            start=(gi == 0),
                                            stop=((pidx == 0 or cb % 2 == 1)
                                                  and gi == len(grps) - 1))
                                    if pidx > 0:
                                        if cb % 2 == 0:
                                            nc.tensor.matmul(
                                                pst[:], idh[:],
                                                h[:, hcol:hcol + 512],
                                                start=False, stop=True)
                                            nc.scalar.copy(
                                                h[:, hcol:hcol + 512], pst[:])
                                        else:
                                            nc.vector.tensor_add(
                                                h[:, hcol:hcol + 512],
                                                h[:, hcol:hcol + 512], pst[:])
                                    elif cb % 2 == 0:
                                        nc.scalar.activation(
                                            h[:, hcol:hcol + 512], pst[:],
                                            AF.Identity, bias=hb[:, m:m + 1])
                                    else:
                                        nc.vector.tensor_scalar_add(
                                            h[:, hcol:hcol + 512], pst[:],
                                            hb[:, m:m + 1])

            with ExitStack() as pctx:
                pop = pctx.enter_context(
                    tc.tile_pool(name=f"po{half}", bufs=4, space="PSUM"))
                psp = pctx.enter_context(
                    tc.tile_pool(name=f"ps{half}", bufs=2, space="PSUM"))
                sp = pctx.enter_context(tc.tile_pool(name=f"sp{half}", bufs=2))
                for cb in range(NB):
                    rr = sp.tile([128, 2048], bf16, tag="r", name="r")
                    nc.vector.tensor_scalar_max(
                        rr[:], h[:, cb * 2048:(cb + 1) * 2048], 0.0)
                    pos = []
                    for p in range(2):
                        pot = pop.tile([128, 512], f32, tag="po", name="po")
                        for q in range(4):
                            nc.tensor.matmul(
                                pot[:],
                                wp2[:, (q * 2 + p) * 128:(q * 2 + p + 1) * 128],
                                rr[:, q * 512:(q + 1) * 512],
                                start=(q == 0), stop=(q == 3))
                        pos.append(pot)
                    ee = sp.tile([128, 1024], f32, tag="e", name="e")
                    for p in range(2):
                        nc.scalar.activation(ee[:, p * 512:(p + 1) * 512],
                                             pos[p][:], AF.Exp,
                                             bias=bp2c[:, p:p + 1])
                    pst = psp.tile([128, 512], f32, tag="ps", name="ps")
                    for p in range(2):
                        nc.tensor.matmul(pst[0:1, :], sumw[:],
                                         ee[:, p * 512:(p + 1) * 512],
                                         start=(p == 0), stop=(p == 1))
                    lss = sp.tile([1, 512], f32, tag="ls", name="ls")
                    nc.scalar.activation(lss[:], pst[0:1, :], AF.Ln)
                    pqt = psp.tile([128, 512], f32, tag="pq", name="pq")
                    nc.tensor.matmul(pqt[:], nones[:], lss[:],
                                     start=True, stop=True)
                    pqs = sp.tile([128, 512], f32, tag="o", name="pqs")
                    nc.vector.tensor_copy(pqs[:], pqt[:])
                    oo2 = sp.tile([128, 1024], f32, tag="e", name="oo2")
                    for p in range(2):
                        nc.vector.scalar_tensor_tensor(
                            oo2[:, p * 512:(p + 1) * 512],
                            pos[p][:], bp2c[:, p:p + 1],
                            pqs[:],
                            op0=ALU.add, op1=ALU.add)
                        c0 = half * VH + cb * 512
                        nc.sync.dma_start(
                            out_d[p * 128:(p + 1) * 128, c0:c0 + 512],
                            oo2[:, p * 512:(p + 1) * 512])

    nc.compile()
    return nc


def _prep_host(inputs):
    """Host-side exact fp32 preprocessing: initial conv, weight packing."""
    x = np.asarray(inputs["x"], np.float32)
    Wc = np.asarray(inputs["Wc"], np.float32)
    bc = np.asarray(inputs["bc"], np.float32)
    Wt = np.asarray(inputs["Wt"], np.float32)
    bt = np.asarray(inputs["bt"], np.float32)
    Ws = np.asarray(inputs["Ws"], np.float32)
    bs = np.asarray(inputs["bs"], np.float32)
    Wskip = np.asarray(inputs["Wskip"], np.float32)
    bskip = np.asarray(inputs["bskip"], np.float32)
    Wdense = np.asarray(inputs["Wdense"], np.float32)
    bdense = np.asarray(inputs["bdense"], np.float32)
    Wp1 = np.asarray(inputs["Wp1"], np.float32)
    bp1 = np.asarray(inputs["bp1"], np.float32)
    Wp2 = np.asarray(inputs["Wp2"], np.float32)
    bp2 = np.asarray(inputs["bp2"], np.float32)

    # initial conv (1 -> 32, k=3, pad=1), exact fp32 on host
    x0 = x[0, 0]
    xp = np.pad(x0, (1, 1))
    x1 = (Wc[:, 0, 0:1] * xp[None, 0:T]
          + Wc[:, 0, 1:2] * xp[None, 1:T + 1]
          + Wc[:, 0, 2:3] * xp[None, 2:T + 2]) + bc[:, None]
    xg = np.pad(x1, ((0, 0), (VOFF, VOFF)))

    # layer-0 tap replicas: row strip 0:32 holds x>>d0, 64:96 holds x<<d0
    d0 = DIL[0]
    xin = np.zeros((NCORES, 96, 2 * WA), BF16)
    for c in range(NCORES):
        for hf in range(2):
            s = c * V + hf * VH
            w = xg[:, s:s + WA].astype(BF16)
            o = hf * WA
            xin[c, RD:2 * RD, o:o + WA] = w
            xin[c, 0:RD, o + d0:o + WA] = w[:, :WA - d0]
            xin[c, 2 * RD:3 * RD, o:o + WA - d0] = w[:, d0:]

    wgk = np.zeros((96, L * 2 * 32), np.float32)
    for l in range(L):
        for f, W in ((0, Wt), (1, Ws)):
            for k in range(3):
                wgk[32 * k:32 * (k + 1),
                    (l * 2 + f) * 32:(l * 2 + f + 1) * 32] = W[l, :, :, k].T

    wdx = np.zeros((128, L * 4 * 64), np.float32)
    for l in range(L):
        for s in range(4):
            for r in range(2):
                wdx[RD * s:RD * (s + 1),
                    (l * 4 + s) * 64 + r * RD:
                    (l * 4 + s) * 64 + (r + 1) * RD] = Wdense[l, :, :, 0].T

    idw = np.zeros((128, 2 * RD), np.float32)
    for r in range(2):
        idw[RD:2 * RD, r * RD:(r + 1) * RD] = np.eye(RD)
    idh = np.eye(128, dtype=np.float32)

    W1s = np.einsum("ab,lbc->lac", Wp1[:, :, 0], Wskip[:, :, :, 0])  # [L,512,32]
    wskp = np.zeros((128, NGRP * 4 * 128), np.float32)
    for G in range(NGRP):
        for m in range(4):
            for jj in range(4):
                l = G * 4 + jj
                if l < L:
                    wskp[32 * jj:32 * (jj + 1),
                         (G * 4 + m) * 128:(G * 4 + m + 1) * 128] = \
                        W1s[l, 128 * m:128 * (m + 1), :].T

    wp2 = np.zeros((128, 8 * 128), np.float32)
    for q in range(4):
        for p in range(2):
            wp2[:, (q * 2 + p) * 128:(q * 2 + p + 1) * 128] = \
                Wp2[128 * p:128 * (p + 1), 128 * q:128 * (q + 1), 0].T

    hbias = Wp1[:, :, 0] @ bskip.sum(axis=0) + bp1     # [512]
    hb = hbias.reshape(4, 128).T.copy()                # [128, 4]

    shared = {
        "wgk": wgk.astype(BF16),
        "wdx": wdx.astype(BF16),
        "idw": idw.astype(BF16),
        "idh": idh.astype(BF16),
        "wskp": wskp.astype(BF16),
        "wp2": wp2.astype(BF16),
        "btt": np.ascontiguousarray(np.tile(bt.T, (4, 1)).astype(np.float32)),
        "bst": np.ascontiguousarray(np.tile(bs.T, (4, 1)).astype(np.float32)),
        "bdc": np.ascontiguousarray(np.tile(bdense.T, (2, 1)).astype(np.float32)),
        "hb": np.ascontiguousarray(hb.astype(np.float32)),
        "bp2c": np.ascontiguousarray(bp2.reshape(2, 128).T.astype(np.float32)),
        "sumw": np.ones((128, 1), np.float32),
        "nones": np.full((1, 128), -1.0, np.float32),
    }
    return xin, shared


def kernel(**inputs):
    from concourse.bass_utils import run_bass_kernel_spmd

    xin, shared = _prep_host(inputs)
    if "nc" not in _cache:
        _cache["nc"] = _build()
    nc = _cache["nc"]

    in_maps = [dict(shared, xin=np.ascontiguousarray(xin[c]))
               for c in range(NCORES)]
    res = run_bass_kernel_spmd(nc, in_maps, core_ids=list(range(NCORES)))

    _last_run["nc"] = nc
    _last_run["in_maps"] = in_maps

    out = np.empty((1, QD, T), np.float32)
    for c in range(NCORES):
        out[0, :, c * V:(c + 1) * V] = res.results[c]["out"]
    return out


# revision 24
# speedup vs baseline: 1.0764x; 1.0764x over previous
"""Trainium2 Bass kernel for a WaveNet-style dilated-conv stack (v3).

Network (per reference):
  x1 = conv1d(x, Wc, bc, d=1, pad=1)                      # 1 -> 32, host-side (exact fp32)
  for l in 27 layers, d = 2^(l%9):
      g = tanh(conv(x, Wt_l, d)) * sigmoid(conv(x, Ws_l, d))   # 32->32, k=3, pad=d
      skip += conv1x1(g, Wskip_l)                              # 32->512
      x = conv1x1(g, Wdense_l) + x
  out = conv1x1(relu(conv1x1(skip, Wp1)), Wp2)            # 512->512->256
  return log_softmax(out, axis=channels)

Device strategy (8 cores, sequence-parallel, no cross-core comm):
  - Wp1 folded into skip weights on host (W1s = Wp1 @ Wskip).
  - Each core owns 16384 steps as 2 halves of 8192 with a 2048-step halo
    (receptive radius 1533 < 2048); window = 12288 computed cols = 6 units
    of 2048.
  - x is stored with its two dilation-shifted tap replicas on partition
    strips {0:32 (x>>d), 32:64 (x), 64:96 (x<<d)}, maintained by SBUF->SBUF
    DMA after each layer's x update.  Each gated conv is then ONE K=96
    matmul per (function, strip): 8 matmuls per 2048-col unit, col-tiled
    M=32 so tanh lands on psum cols 0:512 / sigmoid on 512:1024 with the 4
    strips stacked over 128 partitions -> 128-lane activations.
  - g = tanh*sig computed stacked [128, 512]; scattered to the flat ring
    by 4 SBUF->SBUF DMAs.
  - Two ring panels of 4 layers each; skip matmuls accumulate 2 groups
    (8 layers) per PSUM tile before one DVE add into h (relu bias folded
    into h's first write; h laid out [128, cb*2048+m*512] so the post
    relu is one 2048-wide op).
  - x update is IN PLACE on strip 32:64: even layers on DVE via
    scalar_tensor_tensor((pd+bdense)+x_old); odd layers residual via
    identity matmul on the PE and ScalarE Identity+bias evacuation.
  - Post: exp biased, partition-sum via ones-matmul, ln, broadcast via
    -1s matmul, final add on DVE with one PSUM operand.
"""

import numpy as np
import ml_dtypes

BF16 = ml_dtypes.bfloat16

DIL = [2 ** i for i in range(9)] * 3
L = len(DIL)            # 27
RD, SD, QD = 32, 512, 256
T = 131072
NCORES = 8
V = T // NCORES         # 16384 per core
VH = V // 2             # 8192 per half
HALO = 1536             # >= 1533 total dilation radius
PAD = 256               # >= max dilation
WH = VH + 2 * HALO      # 11264 computed window per half
WA = WH + 2 * PAD       # 11776 allocated width per half
VOFF = HALO + PAD       # 1792 valid-region offset inside window
# units: 5 full 2048-col units + one 1024-col tail (strip-aligned)
UNITS = [(PAD + 2048 * k, 2048, 4) for k in range(5)] + [(PAD + 10240, 1024, 2)]
SV0 = (VOFF - PAD) // 512        # first valid 512-strip index (3)
SV1 = SV0 + VH // 512 - 1        # last valid strip index (18)
NB = VH // 512          # 16 valid 512-col blocks per half
NGRP = (L + 3) // 4     # 7 groups of (up to) 4 layers

_cache = {}
_last_run = {}


def _build():
    from contextlib import ExitStack

    import concourse.bacc as bacc
    import concourse.mybir as mybir
    import concourse.tile as tile

    dt = mybir.dt
    AF = mybir.ActivationFunctionType
    ALU = mybir.AluOpType
    f32, bf16 = dt.float32, dt.bfloat16

    nc = bacc.Bacc("TRN2", target_bir_lowering=False, debug=False,
                   num_devices=NCORES)

    def din(name, shape, dty):
        return nc.dram_tensor(name, shape, dty, kind="ExternalInput").ap()

    xin_d = din("xin", [96, 2 * WA], bf16)
    wgk_d = din("wgk", [96, L * 2 * 32], bf16)   # 3-tap lhsT per (l, fn)
    wdx_d = din("wdx", [128, L * 4 * 64], bf16)  # dense lhsT per (l, strip), M=64
    idw_d = din("idw", [128, 2 * RD], bf16)      # residual identity lhsT, M=64
    idh_d = din("idh", [128, 128], bf16)         # h-accumulate identity lhsT
    wskp_d = din("wskp", [128, NGRP * 4 * 128], bf16)  # skip lhsT per (grp, m)
    wp2_d = din("wp2", [128, 8 * 128], bf16)     # Wp2 lhsT per (q, p)
    btt_d = din("btt", [128, L], f32)            # bt stacked x4
    bst_d = din("bst", [128, L], f32)            # bs stacked x4
    bdc_d = din("bdc", [64, L], f32)             # bdense (2 strips)
    hb_d = din("hb", [128, 4], f32)
    bp2c_d = din("bp2c", [128, 2], f32)
    sumw_d = din("sumw", [128, 1], f32)
    nones_d = din("nones", [1, 128], f32)
    out_d = nc.dram_tensor("out", [QD, V], f32, kind="ExternalOutput").ap()

    with tile.TileContext(nc) as tc, ExitStack() as top:
        wp = top.enter_context(tc.tile_pool(name="wp", bufs=1))

        def load(d, tag):
            t = wp.tile(list(d.shape), d.dtype, tag=tag, name=tag)
            nc.sync.dma_start(t[:], d[:])
            return t

        wgk = load(wgk_d, "wgk")
        wdx = load(wdx_d, "wdx")
        idw = load(idw_d, "idw")
        idh = load(idh_d, "idh")
        wskp = load(wskp_d, "wskp")
        wp2 = load(wp2_d, "wp2")
        btt = load(btt_d, "btt")
        bst = load(bst_d, "bst")
        bdc = load(bdc_d, "bdc")
        hb = load(hb_d, "hb")
        bp2c = load(bp2c_d, "bp2c")
        sumw = load(sumw_d, "sumw")
        nones = load(nones_d, "nones")

        # x + shifted tap replicas; rows 96:128 stay zero for K=128 matmuls
        xx = wp.tile([128, WA], bf16, tag="xx", name="xx")
        ringA = wp.tile([128, WA], bf16, tag="ringA", name="ringA")
        ringB = wp.tile([128, WA], bf16, tag="ringB", name="ringB")
        rings = [ringA, ringB]
        h = wp.tile([128, 4 * VH], bf16, tag="h", name="h")
        nc.vector.memset(xx[:], 0.0)
        nc.gpsimd.memset(ringA[:], 0.0)
        nc.gpsimd.memset(ringB[:], 0.0)

        for half in range(2):
            nc.sync.dma_start(xx[0:96, :], xin_d[:, half * WA:(half + 1) * WA])
            with ExitStack() as lctx:
                pg = lctx.enter_context(
                    tc.tile_pool(name=f"pg{half}", bufs=2, space="PSUM"))
                pd = lctx.enter_context(
                    tc.tile_pool(name=f"pd{half}", bufs=2, space="PSUM"))
                tu = lctx.enter_context(tc.tile_pool(name=f"tu{half}", bufs=4))

                # software pipeline: the dense conv + x update + replica DMAs
                # of unit (l, u) are emitted AFTER unit (l, u+1)'s gated
                # front, so no engine queue head-of-line blocks on the
                # gated->act->mul->scatter chain of the same unit.
                pending = []

                def emit_back(l2, b2, w_, gm2):
                    if l2 == L - 1:
                        return      # x_{27} is dead: skip dense + update
                    on_dve = ((b2 - PAD) // 2048 + l2) % 2 == 0
                    for v2 in range(w_ // 1024):
                        c0 = b2 + v2 * 1024
                        # rows 32:64 of a [64,...] psum tile so the DVE stt
                        # sees matching start partitions
                        pdt = pd.tile([64, 1024], f32, tag="pd", name="pd")
                        for w2 in range(2):
                            s2 = v2 * 2 + w2
                            nc.tensor.matmul(
                                pdt[0:64, w2 * 512:(w2 + 1) * 512],
                                wdx[:, (l2 * 4 + s2) * 64:
                                    (l2 * 4 + s2 + 1) * 64],
                                gm2[:, 0:512],
                                start=True, stop=on_dve,
                                tile_position=(0, 0))
                        if not on_dve:
                            for w2 in range(2):
                                nc.tensor.matmul(
                                    pdt[0:64, w2 * 512:(w2 + 1) * 512],
                                    idw[:],
                                    xx[:, c0 + w2 * 512:c0 + (w2 + 1) * 512],
                                    start=False, stop=True,
                                    tile_position=(0, 0))
                            nc.scalar.activation(
                                xx[RD:2 * RD, c0:c0 + 1024],
                                pdt[RD:2 * RD, :],
                                AF.Identity, bias=bdc[RD:2 * RD, l2:l2 + 1])
                        else:
                            nc.vector.scalar_tensor_tensor(
                                xx[RD:2 * RD, c0:c0 + 1024],
                                pdt[RD:2 * RD, :],
                                bdc[RD:2 * RD, l2:l2 + 1],
                                xx[RD:2 * RD, c0:c0 + 1024],
                                op0=ALU.add, op1=ALU.add)
                    dn = DIL[l2 + 1]
                    nc.sync.dma_start(
                        xx[0:RD, b2 + dn:b2 + w_ + dn],
                        xx[RD:2 * RD, b2:b2 + w_])
                    nc.gpsimd.dma_start(
                        xx[2 * RD:3 * RD, b2 - dn:b2 + w_ - dn],
                        xx[RD:2 * RD, b2:b2 + w_])

                for l in range(L):
                    j = l % 4
                    G = l // 4
                    ring = rings[G % 2]
                    for b0, w, ns in UNITS:
                        pgt = pg.tile([128, 1024], f32, tag="pg", name="pg")
                        for s in range(ns):
                            c0 = b0 + s * 512
                            for f in range(2):
                                nc.tensor.matmul(
                                    pgt[32 * s:32 * (s + 1),
                                        512 * f:512 * f + 512],
                                    wgk[0:96,
                                        (l * 2 + f) * 32:(l * 2 + f + 1) * 32],
                                    xx[0:96, c0:c0 + 512],
                                    start=True, stop=True,
                                    tile_position=(0, 32 * s))
                        pp = 32 * ns
                        gt = tu.tile([128, 512], bf16, tag="gt", name="gt")
                        gs = tu.tile([128, 512], bf16, tag="gs", name="gs")
                        gm = tu.tile([128, 512], bf16, tag="gm", name="gm")
                        nc.scalar.activation(gt[0:pp, :], pgt[0:pp, 0:512],
                                             AF.Tanh, bias=btt[0:pp, l:l + 1])
                        nc.scalar.activation(gs[0:pp, :], pgt[0:pp, 512:1024],
                                             AF.Sigmoid,
                                             bias=bst[0:pp, l:l + 1])
                        nc.vector.tensor_mul(gm[0:pp, :], gt[0:pp, :],
                                             gs[0:pp, :])
                        for s in range(ns):
                            gsi = (b0 - PAD) // 512 + s
                            if SV0 <= gsi <= SV1:
                                nc.vector.tensor_copy(
                                    ring[RD * j:RD * (j + 1),
                                         b0 + 512 * s:b0 + 512 * (s + 1)],
                                    gm[32 * s:32 * (s + 1), :])
                        pending.append((l, b0, w, gm))
                        if len(pending) > 2:
                            emit_back(*pending.pop(0))

                    if (j == 3 and G % 2 == 1) or l == L - 1:
                        while pending:
                            emit_back(*pending.pop(0))
                    if j == 3 or l == L - 1:
                        pidx = G // 2
                        if G % 2 == 1 or l == L - 1:
                            gA = 2 * pidx
                            grps = [g for g in (gA, gA + 1) if g <= G]
                            for m in range(4):
                                for cb in range(NB):
                                    c0 = VOFF + cb * 512
                                    pst = pg.tile([128, 512], f32, tag="pg",
                                                  name="pg")
                                    hcol = cb * 2048 + m * 512
                                    for gi, g in enumerate(grps):
                                        nc.tensor.matmul(
                                            pst[:],
                                            wskp[:, (g * 4 + m) * 128:
                                                 (g * 4 + m + 1) * 128],
                                            rings[g % 2][:, c0:c0 + 512],
                                            start=(gi == 0),
                                            stop=(pidx == 0
                                                  and gi == len(grps) - 1))
                                    if pidx > 0:
                                        nc.tensor.matmul(
                                            pst[:], idh[:],
                                            h[:, hcol:hcol + 512],
                                            start=False, stop=True)
                                        if cb % 2 == 0:
                                            nc.scalar.copy(
                                                h[:, hcol:hcol + 512], pst[:])
                                        else:
                                            nc.vector.tensor_copy(
                                                h[:, hcol:hcol + 512], pst[:])
                                    elif cb % 2 == 0:
                                        nc.scalar.activation(
                                            h[:, hcol:hcol + 512], pst[:],
                                            AF.Identity, bias=hb[:, m:m + 1])
                                    else:
                                        nc.vector.tensor_scalar_add(
                                            h[:, hcol:hcol + 512], pst[:],
                                            hb[:, m:m + 1])

            with ExitStack() as pctx:
                pop = pctx.enter_context(
                    tc.tile_pool(name=f"po{half}", bufs=4, space="PSUM"))
                psp = pctx.enter_context(
                    tc.tile_pool(name=f"ps{half}", bufs=2, space="PSUM"))
                sp = pctx.enter_context(tc.tile_pool(name=f"sp{half}", bufs=2))
                for cb in range(NB):
                    rr = sp.tile([128, 2048], bf16, tag="r", name="r")
                    nc.vector.tensor_scalar_max(
                        rr[:], h[:, cb * 2048:(cb + 1) * 2048], 0.0)
                    pos = []
                    for p in range(2):
                        pot = pop.tile([128, 512], f32, tag="po", name="po")
                        for q in range(4):
                            nc.tensor.matmul(
                                pot[:],
                                wp2[:, (q * 2 + p) * 128:(q * 2 + p + 1) * 128],
                                rr[:, q * 512:(q + 1) * 512],
                                start=(q == 0), stop=(q == 3))
                        pos.append(pot)
                    ee = sp.tile([128, 1024], f32, tag="e", name="e")
                    for p in range(2):
                        nc.scalar.activation(ee[:, p * 512:(p + 1) * 512],
                                             pos[p][:], AF.Exp,
                                             bias=bp2c[:, p:p + 1])
                    pst = psp.tile([128, 512], f32, tag="ps", name="ps")
                    for p in range(2):
                        nc.tensor.matmul(pst[0:1, :], sumw[:],
                                         ee[:, p * 512:(p + 1) * 512],
                                         start=(p == 0), stop=(p == 1))
                    lss = sp.tile([1, 512], f32, tag="ls", name="ls")
                    nc.scalar.activation(lss[:], pst[0:1, :], AF.Ln)
                    pqt = psp.tile([128, 512], f32, tag="pq", name="pq")
                    nc.tensor.matmul(pqt[:], nones[:], lss[:],
                                     start=True, stop=True)
                    pqs = sp.tile([128, 512], f32, tag="o", name="pqs")
                    nc.vector.tensor_copy(pqs[:], pqt[:])
                    oo2 = sp.tile([128, 1024], f32, tag="e", name="oo2")
                    for p in range(2):
                        nc.vector.scalar_tensor_tensor(
                            oo2[:, p * 512:(p + 1) * 512],
                            pos[p][:], bp2c[:, p:p + 1],
                            pqs[:],
                            op0=ALU.add, op1=ALU.add)
                        c0 = half * VH + cb * 512
                        nc.sync.dma_start(
                            out_d[p * 128:(p + 1) * 128, c0:c0 + 512],
                            oo2[:, p * 512:(p + 1) * 512])

    nc.compile()
    return nc


def _prep_host(inputs):
    """Host-side exact fp32 preprocessing: initial conv, weight packing."""
    x = np.asarray(inputs["x"], np.float32)
    Wc = np.asarray(inputs["Wc"], np.float32)
    bc = np.asarray(inputs["bc"], np.float32)
    Wt = np.asarray(inputs["Wt"], np.float32)
    bt = np.asarray(inputs["bt"], np.float32)
    Ws = np.asarray(inputs["Ws"], np.float32)
    bs = np.asarray(inputs["bs"], np.float32)
    Wskip = np.asarray(inputs["Wskip"], np.float32)
    bskip = np.asarray(inputs["bskip"], np.float32)
    Wdense = np.asarray(inputs["Wdense"], np.float32)
    bdense = np.asarray(inputs["bdense"], np.float32)
    Wp1 = np.asarray(inputs["Wp1"], np.float32)
    bp1 = np.asarray(inputs["bp1"], np.float32)
    Wp2 = np.asarray(inputs["Wp2"], np.float32)
    bp2 = np.asarray(inputs["bp2"], np.float32)

    # initial conv (1 -> 32, k=3, pad=1), exact fp32 on host
    x0 = x[0, 0]
    xp = np.pad(x0, (1, 1))
    x1 = (Wc[:, 0, 0:1] * xp[None, 0:T]
          + Wc[:, 0, 1:2] * xp[None, 1:T + 1]
          + Wc[:, 0, 2:3] * xp[None, 2:T + 2]) + bc[:, None]
    xg = np.pad(x1, ((0, 0), (VOFF, VOFF)))

    # layer-0 tap replicas: row strip 0:32 holds x>>d0, 64:96 holds x<<d0
    d0 = DIL[0]
    xin = np.zeros((NCORES, 96, 2 * WA), BF16)
    for c in range(NCORES):
        for hf in range(2):
            s = c * V + hf * VH
            w = xg[:, s:s + WA].astype(BF16)
            o = hf * WA
            xin[c, RD:2 * RD, o:o + WA] = w
            xin[c, 0:RD, o + d0:o + WA] = w[:, :WA - d0]
            xin[c, 2 * RD:3 * RD, o:o + WA - d0] = w[:, d0:]

    wgk = np.zeros((96, L * 2 * 32), np.float32)
    for l in range(L):
        for f, W in ((0, Wt), (1, Ws)):
            for k in range(3):
                wgk[32 * k:32 * (k + 1),
                    (l * 2 + f) * 32:(l * 2 + f + 1) * 32] = W[l, :, :, k].T

    wdx = np.zeros((128, L * 4 * 64), np.float32)
    for l in range(L):
        for s in range(4):
            for r in range(2):
                wdx[RD * s:RD * (s + 1),
                    (l * 4 + s) * 64 + r * RD:
                    (l * 4 + s) * 64 + (r + 1) * RD] = Wdense[l, :, :, 0].T

    idw = np.zeros((128, 2 * RD), np.float32)
    for r in range(2):
        idw[RD:2 * RD, r * RD:(r + 1) * RD] = np.eye(RD)
    idh = np.eye(128, dtype=np.float32)

    W1s = np.einsum("ab,lbc->lac", Wp1[:, :, 0], Wskip[:, :, :, 0])  # [L,512,32]
    wskp = np.zeros((128, NGRP * 4 * 128), np.float32)
    for G in range(NGRP):
        for m in range(4):
            for jj in range(4):
                l = G * 4 + jj
                if l < L:
                    wskp[32 * jj:32 * (jj + 1),
                         (G * 4 + m) * 128:(G * 4 + m + 1) * 128] = \
                        W1s[l, 128 * m:128 * (m + 1), :].T

    wp2 = np.zeros((128, 8 * 128), np.float32)
    for q in range(4):
        for p in range(2):
            wp2[:, (q * 2 + p) * 128:(q * 2 + p + 1) * 128] = \
                Wp2[128 * p:128 * (p + 1), 128 * q:128 * (q + 1), 0].T

    hbias = Wp1[:, :, 0] @ bskip.sum(axis=0) + bp1     # [512]
    hb = hbias.reshape(4, 128).T.copy()                # [128, 4]

    shared = {
        "wgk": wgk.astype(BF16),
        "wdx": wdx.astype(BF16),
        "idw": idw.astype(BF16),
        "idh": idh.astype(BF16),
        "wskp": wskp.astype(BF16),
        "wp2": wp2.astype(BF16),
        "btt": np.ascontiguousarray(np.tile(bt.T, (4, 1)).astype(np.float32)),
        "bst": np.ascontiguousarray(np.tile(bs.T, (4, 1)).astype(np.float32)),
        "bdc": np.ascontiguousarray(np.tile(bdense.T, (2, 1)).astype(np.float32)),
        "hb": np.ascontiguousarray(hb.astype(np.float32)),
        "bp2c": np.ascontiguousarray(bp2.reshape(2, 128).T.astype(np.float32)),
        "sumw": np.ones((128, 1), np.float32),
        "nones": np.full((1, 128), -1.0, np.float32),
    }
    return xin, shared


def kernel(**inputs):
    from concourse.bass_utils import run_bass_kernel_spmd

    xin, shared = _prep_host(inputs)
    if "nc" not in _cache:
        _cache["nc"] = _build()
    nc = _cache["nc"]

    in_maps = [dict(shared, xin=np.ascontiguousarray(xin[c]))
               for c in range(NCORES)]
    res = run_bass_kernel_spmd(nc, in_maps, core_ids=list(range(NCORES)))

    _last_run["nc"] = nc
    _last_run["in_maps"] = in_maps

    out = np.empty((1, QD, T), np.float32)
    for c in range(NCORES):
        out[0, :, c * V:(c + 1) * V] = res.results[c]["out"]
    return out


# revision 26
# speedup vs baseline: 1.1403x; 1.0594x over previous
"""Trainium2 Bass kernel for a WaveNet-style dilated-conv stack (v3).

Network (per reference):
  x1 = conv1d(x, Wc, bc, d=1, pad=1)                      # 1 -> 32, host-side (exact fp32)
  for l in 27 layers, d = 2^(l%9):
      g = tanh(conv(x, Wt_l, d)) * sigmoid(conv(x, Ws_l, d))   # 32->32, k=3, pad=d
      skip += conv1x1(g, Wskip_l)                              # 32->512
      x = conv1x1(g, Wdense_l) + x
  out = conv1x1(relu(conv1x1(skip, Wp1)), Wp2)            # 512->512->256
  return log_softmax(out, axis=channels)

Device strategy (8 cores, sequence-parallel, no cross-core comm):
  - Wp1 folded into skip weights on host (W1s = Wp1 @ Wskip).
  - Each core owns 16384 steps as 2 halves of 8192 with a 1536-step halo
    (receptive radius 1533); window = 11264 computed cols = 5 units of
    2048 + one 1024 tail unit.
  - x is stored with its two dilation-shifted tap replicas on partition
    strips {0:32 (x>>d), 32:64 (x), 64:96 (x<<d)}, maintained by SBUF->SBUF
    DMA after each layer's x update (issue split over the Sync and GpSimd
    queues).  Each gated conv is then ONE K=96 matmul per (function,
    strip): 8 matmuls per 2048-col unit, col-tiled M=32 so tanh lands on
    psum cols 0:512 / sigmoid on 512:1024 with the 4 strips stacked over
    128 partitions -> 128-lane activations.  The dense k=96xM=32 shapes
    keep the PE HAM un-throttled (2.4 GHz) unlike the baseline's K=32
    stream which ran at 1.2 GHz throughout.
  - g = tanh*sig computed stacked [128, 512] on DVE; the flat ring copy
    (DVE 4x bf16) is done only for the 16 valid 512-col strips; the dense
    conv instead reads the STACKED gm tile with per-strip weights.
  - Two ring panels of 4 layers each; skip matmuls accumulate 2 groups
    (8 layers) per PSUM tile, fold the previous h in with an identity
    matmul, and evacuate on ScalarE/DVE alternating by cb (relu bias
    folded into h's first write; h laid out [128, cb*2048+m*512] so the
    post relu is one 2048-wide op).
  - The dense conv + x update + replica DMAs of unit u are emitted two
    units behind the gated front (software pipelining) so no in-order
    engine queue head-of-line blocks on the same unit's chain.
  - x update is IN PLACE on strip 32:64, alternating per (layer, unit):
    DVE scalar_tensor_tensor((pd+bdense)+x_old) or PE identity-matmul
    residual + ScalarE Identity+bias evacuation.
  - Post: relu on DVE (4x bf16), exp biased, partition-sum via ones-
    matmul, ln, -1s-matmul broadcast, final adds on DVE with one PSUM
    operand.  Last layer's dense conv/x update are dead code and skipped.
"""

import numpy as np
import ml_dtypes

BF16 = ml_dtypes.bfloat16

DIL = [2 ** i for i in range(9)] * 3
L = len(DIL)            # 27
RD, SD, QD = 32, 512, 256
T = 131072
NCORES = 8
V = T // NCORES         # 16384 per core
VH = V // 2             # 8192 per half
HALO = 1536             # >= 1533 total dilation radius
PAD = 256               # >= max dilation
WH = VH + 2 * HALO      # 11264 computed window per half
WA = WH + 2 * PAD       # 11776 allocated width per half
VOFF = HALO + PAD       # 1792 valid-region offset inside window
# units: 5 full 2048-col units + one 1024-col tail (strip-aligned)
UNITS = [(PAD + 2048 * k, 2048, 4) for k in range(5)] + [(PAD + 10240, 1024, 2)]
SV0 = (VOFF - PAD) // 512        # first valid 512-strip index (3)
SV1 = SV0 + VH // 512 - 1        # last valid strip index (18)
NB = VH // 512          # 16 valid 512-col blocks per half
NGRP = (L + 3) // 4     # 7 groups of (up to) 4 layers

_cache = {}
_last_run = {}


def _build():
    from contextlib import ExitStack

    import concourse.bacc as bacc
    import concourse.mybir as mybir
    import concourse.tile as tile

    dt = mybir.dt
    AF = mybir.ActivationFunctionType
    ALU = mybir.AluOpType
    f32, bf16 = dt.float32, dt.bfloat16

    nc = bacc.Bacc("TRN2", target_bir_lowering=False, debug=False,
                   num_devices=NCORES)

    def din(name, shape, dty):
        return nc.dram_tensor(name, shape, dty, kind="ExternalInput").ap()

    xin_d = din("xin", [96, 2 * WA], bf16)
    wgk_d = din("wgk", [96, L * 2 * 32], bf16)   # 3-tap lhsT per (l, fn)
    wdx_d = din("wdx", [128, L * 4 * 64], bf16)  # dense lhsT per (l, strip), M=64
    idw_d = din("idw", [128, 2 * RD], bf16)      # residual identity lhsT, M=64
    idh_d = din("idh", [128, 128], bf16)         # h-accumulate identity lhsT
    wskp_d = din("wskp", [128, NGRP * 4 * 128], bf16)  # skip lhsT per (grp, m)
    wp2_d = din("wp2", [128, 8 * 128], bf16)     # Wp2 lhsT per (q, p)
    btt_d = din("btt", [128, L], f32)            # bt stacked x4
    bst_d = din("bst", [128, L], f32)            # bs stacked x4
    bdc_d = din("bdc", [64, L], f32)             # bdense (2 strips)
    hb_d = din("hb", [128, 4], f32)
    bp2c_d = din("bp2c", [128, 2], f32)
    sumw_d = din("sumw", [128, 1], f32)
    nones_d = din("nones", [1, 128], f32)
    out_d = nc.dram_tensor("out", [QD, V], f32, kind="ExternalOutput").ap()

    with tile.TileContext(nc) as tc, ExitStack() as top:
        wp = top.enter_context(tc.tile_pool(name="wp", bufs=1))

        def load(d, tag):
            t = wp.tile(list(d.shape), d.dtype, tag=tag, name=tag)
            nc.sync.dma_start(t[:], d[:])
            return t

        wgk = load(wgk_d, "wgk")
        wdx = load(wdx_d, "wdx")
        idw = load(idw_d, "idw")
        idh = load(idh_d, "idh")
        wskp = load(wskp_d, "wskp")
        wp2 = load(wp2_d, "wp2")
        btt = load(btt_d, "btt")
        bst = load(bst_d, "bst")
        bdc = load(bdc_d, "bdc")
        hb = load(hb_d, "hb")
        bp2c = load(bp2c_d, "bp2c")
        sumw = load(sumw_d, "sumw")
        nones = load(nones_d, "nones")

        # x + shifted tap replicas; rows 96:128 stay zero for K=128 matmuls
        xx = wp.tile([128, WA], bf16, tag="xx", name="xx")
        ringA = wp.tile([128, WA], bf16, tag="ringA", name="ringA")
        ringB = wp.tile([128, WA], bf16, tag="ringB", name="ringB")
        rings = [ringA, ringB]
        h = wp.tile([128, 4 * VH], bf16, tag="h", name="h")
        nc.gpsimd.memset(xx[:], 0.0)
        nc.gpsimd.memset(ringA[:], 0.0)
        nc.gpsimd.memset(ringB[:], 0.0)

        for half in range(2):
            nc.sync.dma_start(xx[0:96, :], xin_d[:, half * WA:(half + 1) * WA])
            with ExitStack() as lctx:
                pg = lctx.enter_context(
                    tc.tile_pool(name=f"pg{half}", bufs=2, space="PSUM"))
                pd = lctx.enter_context(
                    tc.tile_pool(name=f"pd{half}", bufs=2, space="PSUM"))
                tu = lctx.enter_context(tc.tile_pool(name=f"tu{half}", bufs=4))

                # software pipeline: the dense conv + x update + replica DMAs
                # of unit (l, u) are emitted AFTER unit (l, u+1)'s gated
                # front, so no engine queue head-of-line blocks on the
                # gated->act->mul->scatter chain of the same unit.
                pending = []

                def emit_back(l2, b2, w_, gm2):
                    if l2 == L - 1:
                        return      # x_{27} is dead: skip dense + update
                    on_dve = ((b2 - PAD) // 2048 + l2) % 2 == 0
                    for v2 in range(w_ // 1024):
                        c0 = b2 + v2 * 1024
                        # rows 32:64 of a [64,...] psum tile so the DVE stt
                        # sees matching start partitions
                        pdt = pd.tile([64, 1024], f32, tag="pd", name="pd")
                        for w2 in range(2):
                            s2 = v2 * 2 + w2
                            nc.tensor.matmul(
                                pdt[0:64, w2 * 512:(w2 + 1) * 512],
                                wdx[:, (l2 * 4 + s2) * 64:
                                    (l2 * 4 + s2 + 1) * 64],
                                gm2[:, 0:512],
                                start=True, stop=on_dve,
                                tile_position=(0, 0))
                        if not on_dve:
                            for w2 in range(2):
                                nc.tensor.matmul(
                                    pdt[0:64, w2 * 512:(w2 + 1) * 512],
                                    idw[:],
                                    xx[:, c0 + w2 * 512:c0 + (w2 + 1) * 512],
                                    start=False, stop=True,
                                    tile_position=(0, 0))
                            nc.scalar.activation(
                                xx[RD:2 * RD, c0:c0 + 1024],
                                pdt[RD:2 * RD, :],
                                AF.Identity, bias=bdc[RD:2 * RD, l2:l2 + 1])
                        else:
                            nc.vector.scalar_tensor_tensor(
                                xx[RD:2 * RD, c0:c0 + 1024],
                                pdt[RD:2 * RD, :],
                                bdc[RD:2 * RD, l2:l2 + 1],
                                xx[RD:2 * RD, c0:c0 + 1024],
                                op0=ALU.add, op1=ALU.add)
                    dn = DIL[l2 + 1]
                    nc.sync.dma_start(
                        xx[0:RD, b2 + dn:b2 + w_ + dn],
                        xx[RD:2 * RD, b2:b2 + w_])
                    nc.gpsimd.dma_start(
                        xx[2 * RD:3 * RD, b2 - dn:b2 + w_ - dn],
                        xx[RD:2 * RD, b2:b2 + w_])

                for l in range(L):
                    j = l % 4
                    G = l // 4
                    ring = rings[G % 2]
                    for b0, w, ns in UNITS:
                        pgt = pg.tile([128, 1024], f32, tag="pg", name="pg")
                        for s in range(ns):
                            c0 = b0 + s * 512
                            for f in range(2):
                                nc.tensor.matmul(
                                    pgt[32 * s:32 * (s + 1),
                                        512 * f:512 * f + 512],
                                    wgk[0:96,
                                        (l * 2 + f) * 32:(l * 2 + f + 1) * 32],
                                    xx[0:96, c0:c0 + 512],
                                    start=True, stop=True,
                                    tile_position=(0, 32 * s))
                        pp = 32 * ns
                        gt = tu.tile([128, 512], bf16, tag="gt", name="gt")
                        gs = tu.tile([128, 512], bf16, tag="gs", name="gs")
                        gm = tu.tile([128, 512], bf16, tag="gm", name="gm")
                        nc.scalar.activation(gt[0:pp, :], pgt[0:pp, 0:512],
                                             AF.Tanh, bias=btt[0:pp, l:l + 1])
                        nc.scalar.activation(gs[0:pp, :], pgt[0:pp, 512:1024],
                                             AF.Sigmoid,
                                             bias=bst[0:pp, l:l + 1])
                        nc.vector.tensor_mul(gm[0:pp, :], gt[0:pp, :],
                                             gs[0:pp, :])
                        for s in range(ns):
                            gsi = (b0 - PAD) // 512 + s
                            if SV0 <= gsi <= SV1:
                                nc.vector.tensor_copy(
                                    ring[RD * j:RD * (j + 1),
                                         b0 + 512 * s:b0 + 512 * (s + 1)],
                                    gm[32 * s:32 * (s + 1), :])
                        pending.append((l, b0, w, gm))
                        if len(pending) > 2:
                            emit_back(*pending.pop(0))

                    if (j == 3 and G % 2 == 1) or l == L - 1:
                        while pending:
                            emit_back(*pending.pop(0))
                    if j == 3 or l == L - 1:
                        pidx = G // 2
                        if G % 2 == 1 or l == L - 1:
                            gA = 2 * pidx
                            grps = [g for g in (gA, gA + 1) if g <= G]
                            for m in range(4):
                                for cb in range(NB):
                                    c0 = VOFF + cb * 512
                                    pool2 = pg if cb % 2 == 0 else pd
                                    tag2 = "pg" if cb % 2 == 0 else "pd"
                                    pst = pool2.tile([128, 512], f32,
                                                     tag=tag2, name=tag2)
                                    hcol = cb * 2048 + m * 512
                                    for gi, g in enumerate(grps):
                                        nc.tensor.matmul(
                                            pst[:],
                                            wskp[:, (g * 4 + m) * 128:
                                                 (g * 4 + m + 1) * 128],
                                            rings[g % 2][:, c0:c0 + 512],
                                            start=(gi == 0),
                                            stop=(pidx == 0
                                                  and gi == len(grps) - 1))
                                    if pidx > 0:
                                        nc.tensor.matmul(
                                            pst[:], idh[:],
                                            h[:, hcol:hcol + 512],
                                            start=False, stop=True)
                                        if cb % 2 == 0:
                                            nc.scalar.copy(
                                                h[:, hcol:hcol + 512], pst[:])
                                        else:
                                            nc.vector.tensor_copy(
                                                h[:, hcol:hcol + 512], pst[:])
                                    elif cb % 2 == 0:
                                        nc.scalar.activation(
                                            h[:, hcol:hcol + 512], pst[:],
                                            AF.Identity, bias=hb[:, m:m + 1])
                                    else:
                                        nc.vector.tensor_scalar_add(
                                            h[:, hcol:hcol + 512], pst[:],
                                            hb[:, m:m + 1])

            with ExitStack() as pctx:
                pop = pctx.enter_context(
                    tc.tile_pool(name=f"po{half}", bufs=4, space="PSUM"))
                psp = pctx.enter_context(
                    tc.tile_pool(name=f"ps{half}", bufs=2, space="PSUM"))
                sp = pctx.enter_context(tc.tile_pool(name=f"sp{half}", bufs=2))
                for cb in range(NB):
                    rr = sp.tile([128, 2048], bf16, tag="r", name="r")
                    nc.vector.tensor_scalar_max(
                        rr[:], h[:, cb * 2048:(cb + 1) * 2048], 0.0)
                    pos = []
                    for p in range(2):
                        pot = pop.tile([128, 512], f32, tag="po", name="po")
                        for q in range(4):
                            nc.tensor.matmul(
                                pot[:],
                                wp2[:, (q * 2 + p) * 128:(q * 2 + p + 1) * 128],
                                rr[:, q * 512:(q + 1) * 512],
                                start=(q == 0), stop=(q == 3))
                        pos.append(pot)
                    ee = sp.tile([128, 1024], f32, tag="e", name="e")
                    for p in range(2):
                        nc.scalar.activation(ee[:, p * 512:(p + 1) * 512],
                                             pos[p][:], AF.Exp,
                                             bias=bp2c[:, p:p + 1])
                    pst = psp.tile([128, 512], f32, tag="ps", name="ps")
                    for p in range(2):
                        nc.tensor.matmul(pst[0:1, :], sumw[:],
                                         ee[:, p * 512:(p + 1) * 512],
                                         start=(p == 0), stop=(p == 1))
                    lss = sp.tile([1, 512], f32, tag="ls", name="ls")
                    nc.scalar.activation(lss[:], pst[0:1, :], AF.Ln)
                    pqt = psp.tile([128, 512], f32, tag="pq", name="pq")
                    nc.tensor.matmul(pqt[:], nones[:], lss[:],
                                     start=True, stop=True)
                    pqs = sp.tile([128, 512], f32, tag="o", name="pqs")
                    nc.vector.tensor_copy(pqs[:], pqt[:])
                    oo2 = sp.tile([128, 1024], f32, tag="e", name="oo2")
                    for p in range(2):
                        nc.vector.scalar_tensor_tensor(
                            oo2[:, p * 512:(p + 1) * 512],
                            pos[p][:], bp2c[:, p:p + 1],
                            pqs[:],
                            op0=ALU.add, op1=ALU.add)
                        c0 = half * VH + cb * 512
                        nc.sync.dma_start(
                            out_d[p * 128:(p + 1) * 128, c0:c0 + 512],
                            oo2[:, p * 512:(p + 1) * 512])

    nc.compile()
    return nc


def _prep_host(inputs):
    """Host-side exact fp32 preprocessing: initial conv, weight packing."""
    x = np.asarray(inputs["x"], np.float32)
    Wc = np.asarray(inputs["Wc"], np.float32)
    bc = np.asarray(inputs["bc"], np.float32)
    Wt = np.asarray(inputs["Wt"], np.float32)
    bt = np.asarray(inputs["bt"], np.float32)
    Ws = np.asarray(inputs["Ws"], np.float32)
    bs = np.asarray(inputs["bs"], np.float32)
    Wskip = np.asarray(inputs["Wskip"], np.float32)
    bskip = np.asarray(inputs["bskip"], np.float32)
    Wdense = np.asarray(inputs["Wdense"], np.float32)
    bdense = np.asarray(inputs["bdense"], np.float32)
    Wp1 = np.asarray(inputs["Wp1"], np.float32)
    bp1 = np.asarray(inputs["bp1"], np.float32)
    Wp2 = np.asarray(inputs["Wp2"], np.float32)
    bp2 = np.asarray(inputs["bp2"], np.float32)

    # initial conv (1 -> 32, k=3, pad=1), exact fp32 on host
    x0 = x[0, 0]
    xp = np.pad(x0, (1, 1))
    x1 = (Wc[:, 0, 0:1] * xp[None, 0:T]
          + Wc[:, 0, 1:2] * xp[None, 1:T + 1]
          + Wc[:, 0, 2:3] * xp[None, 2:T + 2]) + bc[:, None]
    xg = np.pad(x1, ((0, 0), (VOFF, VOFF)))

    # layer-0 tap replicas: row strip 0:32 holds x>>d0, 64:96 holds x<<d0
    d0 = DIL[0]
    xin = np.zeros((NCORES, 96, 2 * WA), BF16)
    for c in range(NCORES):
        for hf in range(2):
            s = c * V + hf * VH
            w = xg[:, s:s + WA].astype(BF16)
            o = hf * WA
            xin[c, RD:2 * RD, o:o + WA] = w
            xin[c, 0:RD, o + d0:o + WA] = w[:, :WA - d0]
            xin[c, 2 * RD:3 * RD, o:o + WA - d0] = w[:, d0:]

    wgk = np.zeros((96, L * 2 * 32), np.float32)
    for l in range(L):
        for f, W in ((0, Wt), (1, Ws)):
            for k in range(3):
                wgk[32 * k:32 * (k + 1),
                    (l * 2 + f) * 32:(l * 2 + f + 1) * 32] = W[l, :, :, k].T

    wdx = np.zeros((128, L * 4 * 64), np.float32)
    for l in range(L):
        for s in range(4):
            for r in range(2):
                wdx[RD * s:RD * (s + 1),
                    (l * 4 + s) * 64 + r * RD:
                    (l * 4 + s) * 64 + (r + 1) * RD] = Wdense[l, :, :, 0].T

    idw = np.zeros((128, 2 * RD), np.float32)
    for r in range(2):
        idw[RD:2 * RD, r * RD:(r + 1) * RD] = np.eye(RD)
    idh = np.eye(128, dtype=np.float32)

    W1s = np.einsum("ab,lbc->lac", Wp1[:, :, 0], Wskip[:, :, :, 0])  # [L,512,32]
    wskp = np.zeros((128, NGRP * 4 * 128), np.float32)
    for G in range(NGRP):
        for m in range(4):
            for jj in range(4):
                l = G * 4 + jj
                if l < L:
                    wskp[32 * jj:32 * (jj + 1),
                         (G * 4 + m) * 128:(G * 4 + m + 1) * 128] = \
                        W1s[l, 128 * m:128 * (m + 1), :].T

    wp2 = np.zeros((128, 8 * 128), np.float32)
    for q in range(4):
        for p in range(2):
            wp2[:, (q * 2 + p) * 128:(q * 2 + p + 1) * 128] = \
                Wp2[128 * p:128 * (p + 1), 128 * q:128 * (q + 1), 0].T

    hbias = Wp1[:, :, 0] @ bskip.sum(axis=0) + bp1     # [512]
    hb = hbias.reshape(4, 128).T.copy()                # [128, 4]

    shared = {
        "wgk": wgk.astype(BF16),
        "wdx": wdx.astype(BF16),
        "idw": idw.astype(BF16),
        "idh": idh.astype(BF16),
        "wskp": wskp.astype(BF16),
        "wp2": wp2.astype(BF16),
        "btt": np.ascontiguousarray(np.tile(bt.T, (4, 1)).astype(np.float32)),
        "bst": np.ascontiguousarray(np.tile(bs.T, (4, 1)).astype(np.float32)),
        "bdc": np.ascontiguousarray(np.tile(bdense.T, (2, 1)).astype(np.float32)),
        "hb": np.ascontiguousarray(hb.astype(np.float32)),
        "bp2c": np.ascontiguousarray(bp2.reshape(2, 128).T.astype(np.float32)),
        "sumw": np.ones((128, 1), np.float32),
        "nones": np.full((1, 128), -1.0, np.float32),
    }
    return xin, shared


def kernel(**inputs):
    from concourse.bass_utils import run_bass_kernel_spmd

    xin, shared = _prep_host(inputs)
    if "nc" not in _cache:
        _cache["nc"] = _build()
    nc = _cache["nc"]

    in_maps = [dict(shared, xin=np.ascontiguousarray(xin[c]))
               for c in range(NCORES)]
    res = run_bass_kernel_spmd(nc, in_maps, core_ids=list(range(NCORES)))

    _last_run["nc"] = nc
    _last_run["in_maps"] = in_maps

    out = np.empty((1, QD, T), np.float32)
    for c in range(NCORES):
        out[0, :, c * V:(c + 1) * V] = res.results[c]["out"]
    return out
